# revision 1
# baseline (speedup 1.0000x reference)
"""Trainium2 Bass kernel for nn_MDSFF (deformable-sampling sparse attention).

Math restructuring (key to making this fast on TRN2):
  - Offsets are tanh-bounded to +-1 px, so bilinear grid-sample == 9-tap
    local stencil with per-pixel weights w_d = relu(1 - |clip(t,lo,hi) - d|),
    zeroed at image edges.
  - 1x1 convs commute with spatial shifts: k_proj(sampled) = sum_t tw_t *
    shift_t(k_w @ x_aux), and the output projection folds the same way, so
    the huge [B,K,C,H,W] sampled tensor is never materialized.
  - sim[k,h] = sum_t tw[k,t] * S_t[h], where S_t[h] = sum_{d in head h}
    q * shift_t(A); S_t is computed once per tap (not per k).
  - final = sum_t G_t * shift_t(out_w @ x_aux), G_t = sum_k tw[k,t]*wk[k].
  - tw never materialized: (wy*wx)*X computed as wy*(wx*X).

Sharding: 8 cores = 4 batches x 2 H-halves. Each core processes its half in
two 16-row column chunks. Host (numpy) does only data movement: slicing,
halo padding, layout, constant masks/selectors.
"""

import sys

sys.path.insert(0, "/opt/trn_rl_repo")

import numpy as np

import concourse.bass as bass
import concourse.mybir as mybir
from concourse import tile
from concourse.bass_utils import run_bass_kernel_spmd

# ---------------- problem constants (hardcoded per contract) ----------------
B, C, H, W = 4, 256, 64, 64
K = 8
NHEADS = 4
NCORES = 8
ROWS = 32         # center rows per core
CHR = 16          # rows per chunk
NCHUNK = 2
N1 = CHR * W      # 1024 center pixels per chunk
HR = 34           # haloed rows per core
XA_W = HR * W     # 2176
XM_W = 2248       # padded x_main width: col = 2 + 66*r + w
AW = 2 + 18 * W   # 1154: per-chunk A/Ao width, data cols [1, 1153)
TAPS = [(dy, dx) for dy in (-1, 0, 1) for dx in (-1, 0, 1)]

F32 = mybir.dt.float32
R32 = mybir.dt.float32r
AF = mybir.ActivationFunctionType
OP = mybir.AluOpType

USE_F32R = True    # float32r tiles: matmul 1 cyc/row vs 4 for fp32
FULL_PROJ = False  # keep conv/q/A/Ao matmuls in fp32 (no observed error gain)

_CACHE = {}


DT = R32 if USE_F32R else F32


def _mmcast(ap):
    return ap


# ============================ program builder ===============================

def _build_program():
    MAX_WAITS = 1

    SPLIT_OK = {
        "InstDrain", "InstNoOp", "InstMatmult", "InstLdweights",
        "InstTensorTensor", "InstActivation", "InstTensorScalarPtr",
        "InstTensorReduce", "InstCopy", "InstMemSet", "InstMemset", "InstReciprocal",
        "InstTensorTensorReduce", "InstTensorCopy",
    }

    def split_waits(nc):
        # walrus in this container rejects instructions carrying more than
        # MAX_WAITS semaphore waits; spill extras onto same-engine nops.
        # Only safe for engine-FIFO instructions: hoisting a DMA descriptor's
        # wait onto the SP sequencer can deadlock (SP stalls instead of the
        # DGE queue, while producers wait on later SP-pushed DMAs).
        f = nc.m.functions[0]
        for bb in f.blocks:
            insts = bb.instructions
            out = []
            changed = False
            for inst in insts:
                si = inst.sync_info
                waits = list(si.on_wait) if si and si.on_wait else []
                if (len(waits) > MAX_WAITS
                        and type(inst).__name__ in SPLIT_OK
                        and all(w.wait_reg is None for w in waits)):
                    changed = True
                    rest, keep = waits[:-MAX_WAITS], waits[-MAX_WAITS:]
                    for i in range(0, len(rest), MAX_WAITS):
                        nop = mybir.InstNoOp(
                            name=f"Wspill_{inst.name}_{i}", ins=[], outs=[])
                        nop.engine = inst.engine
                        nop.sync_info = mybir.SyncInfo(
                            on_wait=rest[i : i + MAX_WAITS], on_update=[])
                        nc.register_instruction(nop)
                        out.append(nop)
                    inst.sync_info = mybir.SyncInfo(
                        on_wait=keep, on_update=list(si.on_update or [])
                    )
                out.append(inst)
            if changed:
                bb.instructions = out

    nc = bass.Bass("TRN2", target_bir_lowering=False, debug=False,
                   num_devices=NCORES)

    dp = nc.dram_tensor
    xm_d = dp("xm", [128, 2, XM_W], DT, kind="ExternalInput")
    xa_d = dp("xa", [128, 2, XA_W], DT, kind="ExternalInput")
    # packed weights: qwT(512) kwT(512) owT(512) cwT(288) sel(64) i128(128)
    wcat_d = dp("wcat", [128, 2016], DT, kind="ExternalInput")
    # packed 32-row consts: i32|hs|avg4|qb (224), sel16 (64), offb (1),
    # dbias (4), lox|hix|mxm|mxp (4*64), loy|hiy|mym|myp (4*32)
    mcat_d = dp("mcat", [32, 677], DT, kind="ExternalInput")
    y_d = dp("y", [128, 2, ROWS * W], DT, kind="ExternalOutput")

    V = nc.vector
    A_ = nc.scalar

    def mm(out, lhsT, rhs, start, stop, full=False):
        if full and USE_F32R and FULL_PROJ:
            lhsT, rhs = lhsT.bitcast(F32), rhs.bitcast(F32)
        nc.tensor.matmul(out=out, lhsT=lhsT, rhs=rhs,
                         start=start, stop=stop, skip_group_check=True)

    with tile.TileContext(nc) as tc:
        with (
            nc.allow_low_precision(reason="float32r tiles: fp22 mantissa "
                                   "rounding is within this kernel's budget"),
            tc.tile_pool(name="pw", bufs=1) as pw,          # weights/selectors
            tc.tile_pool(name="pio", bufs=1) as pio,        # xm, xa
            tc.tile_pool(name="pbig", bufs=1) as pbig,      # q, A, Ao, out
            tc.tile_pool(name="pmap", bufs=1) as pmap,      # 32-row maps
            tc.tile_pool(name="pm", bufs=1) as pm,          # big S-stage temps
            tc.tile_pool(name="psum", bufs=1, space="PSUM") as psp,
        ):
            xm = pio.tile([128, 2, XM_W], DT, tag="xm")
            xa = pio.tile([128, 2, XA_W], DT, tag="xa")
            for cb in range(2):
                nc.sync.dma_start(out=xm[:, cb, :], in_=xm_d[:, cb, :])
            for cb in range(2):
                nc.sync.dma_start(out=xa[:, cb, :], in_=xa_d[:, cb, :])

            wcat = pw.tile([128, 2016], DT, tag="wcat")
            mcat = pw.tile([32, 677], DT, tag="mcat")
            nc.sync.dma_start(out=wcat[:], in_=wcat_d[:])
            nc.sync.dma_start(out=mcat[:], in_=mcat_d[:])

            def w4(o):  # [128, 2, 2, 128] block at col o
                return wcat[:, o : o + 512].rearrange(
                    "p (cb ob m) -> p cb ob m", cb=2, ob=2)

            qwT, kwT, owT = w4(0), w4(512), w4(1024)
            cwT = wcat[:, 1536:1824].rearrange("p (t cb m) -> p t cb m", t=9, cb=2)
            sel = wcat[:, 1824:1888].rearrange("p (cb j) -> p cb j", cb=2)
            i128 = wcat[:, 1888:2016]
            i32 = mcat[:, 0:32]
            hs = mcat[:, 32:64]
            avg4 = mcat[:, 64:96]
            qb = mcat[:, 96:224]
            sel16 = mcat[0:16, 224:288]
            offb = mcat[0:16, 288:289]
            dbias = mcat[:, 289:293]

            def xconst(i):   # [32, 16, 64] broadcast of a 64-wide column const
                return (mcat[:, None, 293 + 64 * i : 357 + 64 * i]
                        .broadcast_to([32, CHR, 64]))

            def yconst(i, ch):  # [32, 16, 64] broadcast of a per-row const
                c0 = 549 + 32 * i + CHR * ch
                return (mcat[:, c0 : c0 + CHR, None]
                        .broadcast_to([32, CHR, 64]))

            def xm_view(cb, row0, dy=0, dx=0):
                # [128, 8, 64] view of padded x_main: 8 rows starting at
                # haloed local row (1 + row0 + dy), w-shift dx.
                o = 2 + 66 * (1 + row0 + dy) + dx
                return (xm[:, cb, o : o + 8 * 66]
                        .rearrange("p (r w) -> p r w", w=66)[:, :, 0:64])

            for ch in range(NCHUNK):
                R0 = CHR * ch

                # ---------------- offset conv + tanh ----------------
                off = pmap.tile([16, N1], DT, tag="off", name=f"off{ch}")
                for i in range(2):
                    pso = psp.tile([128, 512], F32, tag="ps512", bufs=2,
                                   name=f"pso{ch}{i}")
                    for cb in range(2):
                        for t in range(9):
                            dy, dx = TAPS[t]
                            mm(pso[0:16, :], cwT[:, t, cb, :],
                               xm_view(cb, R0 + 8 * i, dy, dx),
                               start=(t == 0 and cb == 0),
                               stop=(t == 8 and cb == 1), full=True)
                    A_.activation(out=off[:, 512 * i : 512 * (i + 1)],
                                  in_=pso[0:16, :], func=AF.Tanh,
                                  bias=offb, scale=1.0)

                # ---------------- q / A / Ao projections ----------------
                q = pbig.tile([128, 2, N1], DT, tag="q", name=f"q{ch}")
                for ob in range(2):
                    for i in range(2):
                        ps = psp.tile([128, 512], F32, tag="ps512", bufs=2,
                                      name=f"psq{ch}{ob}{i}")
                        for cb in range(2):
                            mm(ps[:], qwT[:, cb, ob, :], xm_view(cb, R0 + 8 * i),
                               start=(cb == 0), stop=(cb == 1), full=True)
                        A_.activation(out=q[:, ob, 512 * i : 512 * (i + 1)],
                                      in_=ps[:], func=AF.Copy)

                AT = pbig.tile([128, 2, AW], DT, tag="A", name=f"A{ch}")
                AoT = pbig.tile([128, 2, AW], DT, tag="Ao", name=f"Ao{ch}")
                for dst in (AT, AoT):
                    V.memset(dst[:, :, 0:1].bitcast(F32), 0.0)
                    V.memset(dst[:, :, AW - 1 : AW].bitcast(F32), 0.0)
                for di, (dst, wT) in enumerate(((AT, kwT), (AoT, owT))):
                    for ob in range(2):
                        for j, sz in ((0, 512), (1, 512), (2, 128)):
                            ps = psp.tile([128, 512], F32, tag="ps512", bufs=2,
                                          name=f"psP{ch}{di}{ob}{j}")
                            rhs = xa[:, :, 64 * R0 + 512 * j : 64 * R0 + 512 * j + sz]
                            for cb in range(2):
                                mm(ps[:, 0:sz], wT[:, cb, ob, :], rhs[:, cb, :],
                                   start=(cb == 0), stop=(cb == 1), full=True)
                            A_.activation(
                                out=dst[:, ob, 1 + 512 * j : 1 + 512 * j + sz],
                                in_=ps[:, 0:sz], func=AF.Copy)

                # ---------------- tap-weight maps ----------------
                # T2[:, 0, :] = tx (replicated over heads), T2[:, 1, :] = ty
                T2 = pmap.tile([32, 2, N1], DT, tag="T2", name=f"T2{ch}")
                for i in range(2):
                    ps64 = psp.tile([128, 512], F32, tag="ps512", bufs=2,
                                    name=f"ps64{ch}{i}")
                    mm(ps64[0:64, :], sel16, off[:, 512 * i : 512 * (i + 1)],
                       start=True, stop=True)
                    sl = slice(512 * i, 512 * (i + 1))
                    A_.activation(out=T2[:, 0, sl], in_=ps64[0:32, :], func=AF.Copy)
                    A_.activation(out=T2[:, 1, sl], in_=ps64[32:64, :], func=AF.Copy)

                txv = T2[:, 0, :].rearrange("p (r w) -> p r w", w=64)
                tyv = T2[:, 1, :].rearrange("p (r w) -> p r w", w=64)
                V.tensor_tensor(out=txv, in0=txv, in1=xconst(0), op=OP.max)
                V.tensor_tensor(out=txv, in0=txv, in1=xconst(1), op=OP.min)
                V.tensor_tensor(out=tyv, in0=tyv, in1=yconst(0, ch), op=OP.max)
                V.tensor_tensor(out=tyv, in0=tyv, in1=yconst(1, ch), op=OP.min)

                WXs, WYs = {}, {}
                for d in (-1, 0, 1):
                    wd = pmap.tile([32, 2, N1], DT, tag=f"wd{d}",
                                   name=f"wd{d}_{ch}")
                    db = dbias[:, d + 1 : d + 2]
                    A_.activation(out=wd[:], in_=T2[:], func=AF.Abs, bias=db)
                    A_.activation(out=wd[:], in_=wd[:], func=AF.Relu,
                                  scale=-1.0, bias=dbias[:, 3:4])
                    if d != 0:
                        mi = 2 if d == -1 else 3
                        wdx = wd[:, 0, :].rearrange("p (r w) -> p r w", w=64)
                        wdy = wd[:, 1, :].rearrange("p (r w) -> p r w", w=64)
                        V.tensor_tensor(out=wdx, in0=wdx, in1=xconst(mi),
                                        op=OP.mult)
                        V.tensor_tensor(out=wdy, in0=wdy, in1=yconst(mi, ch),
                                        op=OP.mult)
                    WXs[d], WYs[d] = wd[:, 0, :], wd[:, 1, :]

                # ---------------- S maps + sim + softmax ----------------
                sim_ps = psp.tile([32, N1], F32, tag="sim", name=f"sim{ch}")
                for t in range(9):
                    dy, dx = TAPS[t]
                    o_t = 65 + 64 * dy + dx
                    M = pm.tile([128, 2, N1], DT, tag="M", bufs=2,
                                name=f"M{ch}{t}")
                    V.tensor_tensor(out=M[:], in0=q[:],
                                    in1=AT[:, :, o_t : o_t + N1], op=OP.mult)
                    S_t = pmap.tile([32, N1], DT, tag="sp", bufs=4,
                                    name=f"S{ch}{t}")
                    for j in range(2):
                        sl = slice(512 * j, 512 * (j + 1))
                        s_ps = psp.tile([128, 512], F32, tag="ps512", bufs=2,
                                        name=f"sps{ch}{t}{j}")
                        for cb in range(2):
                            mm(s_ps[0:32, :], sel[:, cb, :], M[:, cb, sl],
                               start=(cb == 0), stop=(cb == 1))
                        A_.activation(out=S_t[:, sl], in_=s_ps[0:32, :],
                                      func=AF.Copy)
                    U_t = pmap.tile([32, N1], DT, tag="sp", bufs=4,
                                    name=f"U{ch}{t}")
                    nc.gpsimd.tensor_tensor(out=U_t[:], in0=WXs[dx][:],
                                            in1=S_t[:], op=OP.mult)
                    P_t = pmap.tile([32, N1], DT, tag="sp", bufs=4,
                                    name=f"P{ch}{t}")
                    V.tensor_tensor(out=P_t[:], in0=WYs[dy][:], in1=U_t[:],
                                    op=OP.mult)
                    for j in range(2):
                        sl = slice(512 * j, 512 * (j + 1))
                        mm(sim_ps[:, sl], i32, P_t[:, sl],
                           start=(t == 0), stop=(t == 8))

                E = pmap.tile([32, N1], DT, tag="sm", bufs=3, name=f"E{ch}")
                A_.activation(out=E[:], in_=sim_ps[:], func=AF.Exp,
                              bias=dbias[:, 1:2], scale=0.125)
                Rr = pmap.tile([32, N1], DT, tag="sm", bufs=3, name=f"R{ch}")
                for j in range(2):
                    sl = slice(512 * j, 512 * (j + 1))
                    d_ps = psp.tile([128, 512], F32, tag="ps512", bufs=2,
                                    name=f"dps{ch}{j}")
                    mm(d_ps[0:32, :], hs, E[:, sl], start=True, stop=True)
                    V.reciprocal(out=Rr[:, sl], in_=d_ps[0:32, :])
                Ff = pmap.tile([32, N1], DT, tag="sm", bufs=3, name=f"F{ch}")
                V.tensor_tensor(out=Ff[:], in0=E[:], in1=Rr[:], op=OP.mult)
                WKt = pmap.tile([32, N1], DT, tag="WK", name=f"WK{ch}")
                for j in range(2):
                    sl = slice(512 * j, 512 * (j + 1))
                    wk_ps = psp.tile([128, 512], F32, tag="ps512", bufs=2,
                                     name=f"wkps{ch}{j}")
                    mm(wk_ps[0:32, :], avg4, Ff[:, sl], start=True, stop=True)
                    A_.activation(out=WKt[:, sl], in_=wk_ps[0:32, :], func=AF.Copy)

                # ---------------- G maps + final combine ----------------
                Vx = {}
                for d in (-1, 0, 1):
                    v = pmap.tile([32, N1], DT, tag=f"v{d}", name=f"v{d}_{ch}")
                    V.tensor_tensor(out=v[:], in0=WXs[d][:], in1=WKt[:],
                                    op=OP.mult)
                    Vx[d] = v

                fin = [psp.tile([128, N1], F32, tag="fin", bufs=2,
                                name=f"fin{ch}{_ob}") for _ob in range(2)]
                for t in range(9):
                    dy, dx = TAPS[t]
                    o_t = 65 + 64 * dy + dx
                    Q_t = pmap.tile([32, N1], DT, tag="qgb", bufs=4,
                                    name=f"Q{ch}{t}")
                    nc.gpsimd.tensor_tensor(out=Q_t[:], in0=WYs[dy][:],
                                            in1=Vx[dx][:], op=OP.mult)
                    Gb = pmap.tile([128, N1], DT, tag="qgb", bufs=4,
                                   name=f"Gb{ch}{t}")
                    for j in range(2):
                        sl = slice(512 * j, 512 * (j + 1))
                        gb_ps = psp.tile([128, 512], F32, tag="ps512", bufs=2,
                                         name=f"gbps{ch}{t}{j}")
                        mm(gb_ps[:], qb, Q_t[:, sl], start=True, stop=True)
                        A_.activation(out=Gb[:, sl], in_=gb_ps[:], func=AF.Copy)
                    for ob in range(2):
                        Fv = pm.tile([128, N1], DT, tag="Fv", bufs=2,
                                     name=f"Fv{ch}{t}{ob}")
                        eng = V if ob == 0 else nc.gpsimd
                        eng.tensor_tensor(out=Fv[:], in0=Gb[:],
                                          in1=AoT[:, ob, o_t : o_t + N1],
                                          op=OP.mult)
                        for j in range(2):
                            sl = slice(512 * j, 512 * (j + 1))
                            mm(fin[ob][:, sl], i128, Fv[:, sl],
                               start=(t == 0), stop=(t == 8))

                out_sb = pbig.tile([128, 2, N1], DT, tag="osb", name=f"osb{ch}")
                for ob in range(2):
                    A_.activation(out=out_sb[:, ob, :], in_=fin[ob][:],
                                  func=AF.Copy)
                nc.gpsimd.dma_start(out=y_d[:, :, N1 * ch : N1 * (ch + 1)],
                                     in_=out_sb[:])

    split_waits(nc)
    return nc


# ============================ host-side prep ===============================

def _consts():
    perm = [2 * k for k in range(K)] + [2 * k + 1 for k in range(K)]

    sel = np.zeros((128, 2, 32), np.float32)
    for cb in range(2):
        for p in range(128):
            h = (128 * cb + p) // 64
            for j in range(32):
                if j % 4 == h:
                    sel[p, cb, j] = 1.0

    sel16 = np.zeros((16, 64), np.float32)
    for j in range(32):
        sel16[j // 4, j] = 1.0           # tx: channel k
        sel16[8 + j // 4, 32 + j] = 1.0  # ty: channel 8+k

    i32 = np.eye(32, dtype=np.float32)
    hs = np.zeros((32, 32), np.float32)
    avg4 = np.zeros((32, 32), np.float32)
    for i in range(32):
        for j in range(32):
            if i % 4 == j % 4:
                hs[i, j] = 1.0
            if i // 4 == j // 4:
                avg4[i, j] = 0.25
    qb = np.full((32, 128), 0.25, np.float32)
    i128 = np.eye(128, dtype=np.float32)
    dbias = np.zeros((32, 4), np.float32)
    dbias[:, 0], dbias[:, 2], dbias[:, 3] = 1.0, -1.0, 1.0
    return perm, sel, sel16, i32, hs, avg4, qb, i128, dbias


def _per_core_consts(h0):
    # x consts [4, 64]: lox, hix, mxm, mxp;  y consts [4, 32]: loy, hiy, mym, myp
    w = np.arange(W, dtype=np.float32)
    g = h0 + np.arange(ROWS, dtype=np.float32)
    xc = np.stack([-0.5 - w, 63.5 - w,
                   (w != 0).astype(np.float32),
                   (w != W - 1).astype(np.float32)])
    yc = np.stack([-0.5 - g, 63.5 - g,
                   (g != 0).astype(np.float32),
                   (g != H - 1).astype(np.float32)])
    return xc.astype(np.float32), yc.astype(np.float32)


def _prep_inputs(x_main, x_aux, offset_w, offset_b, q_w, k_w, out_w):
    perm, sel, sel16, i32, hs, avg4, qb, i128, dbias = _consts()

    def wT(wmat):
        # [128, 2, 2, 128]: lhsT[cin_local, cb, ob, o_local] = w[o, cin]
        r = np.zeros((128, 2, 2, 128), np.float32)
        for cb in range(2):
            for ob in range(2):
                r[:, cb, ob, :] = wmat[128 * ob : 128 * (ob + 1),
                                       128 * cb : 128 * (cb + 1)].T
        return r

    wperm = offset_w[perm]           # [16, C, 3, 3]
    bperm = offset_b[perm].astype(np.float32)
    cwT = np.zeros((128, 9, 2, 16), np.float32)
    for t, (dy, dx) in enumerate(TAPS):
        for cb in range(2):
            cwT[:, t, cb, :] = wperm[:, 128 * cb : 128 * (cb + 1),
                                     dy + 1, dx + 1].T

    wcat = np.zeros((128, 2016), np.float32)
    wcat[:, 0:512] = wT(q_w).reshape(128, 512)
    wcat[:, 512:1024] = wT(k_w).reshape(128, 512)
    wcat[:, 1024:1536] = wT(out_w).reshape(128, 512)
    wcat[:, 1536:1824] = cwT.reshape(128, 288)
    wcat[:, 1824:1888] = sel.reshape(128, 64)
    wcat[:, 1888:2016] = i128

    mcat0 = np.zeros((32, 677), np.float32)
    mcat0[:, 0:32] = i32
    mcat0[:, 32:64] = hs
    mcat0[:, 64:96] = avg4
    mcat0[:, 96:224] = qb
    mcat0[0:16, 224:288] = sel16
    mcat0[0:16, 288] = bperm
    mcat0[:, 289:293] = dbias

    in_maps = []
    for core in range(NCORES):
        b, half = core // 2, core % 2
        h0 = ROWS * half
        xm = np.zeros((128, 2, XM_W), np.float32)
        xa = np.zeros((128, 2, XA_W), np.float32)
        for r in range(HR):
            g = h0 - 1 + r
            if 0 <= g < H:
                for cb in range(2):
                    xm[:, cb, 2 + 66 * r : 2 + 66 * r + 64] = \
                        x_main[b, 128 * cb : 128 * (cb + 1), g, :]
                    xa[:, cb, 64 * r : 64 * r + 64] = \
                        x_aux[b, 128 * cb : 128 * (cb + 1), g, :]
        xc, yc = _per_core_consts(h0)
        mcat = mcat0.copy()
        for i in range(4):
            mcat[:, 293 + 64 * i : 357 + 64 * i] = xc[i][None, :]
            mcat[:, 549 + 32 * i : 581 + 32 * i] = yc[i][None, :]
        in_maps.append(dict(xm=xm, xa=xa, wcat=wcat, mcat=mcat))
    return in_maps


def kernel(**inputs):
    inputs = {k: np.asarray(v, dtype=np.float32) for k, v in inputs.items()}
    if "nc" not in _CACHE:
        _CACHE["nc"] = _build_program()
    nc = _CACHE["nc"]
    in_maps = _prep_inputs(
        inputs["x_main"], inputs["x_aux"], inputs["offset_w"],
        inputs["offset_b"], inputs["q_w"], inputs["k_w"], inputs["out_w"])
    res = run_bass_kernel_spmd(nc, in_maps, list(range(NCORES))).results

    out = np.zeros((B, C, H, W), np.float32)
    for core in range(NCORES):
        b, half = core // 2, core % 2
        y = res[core]["y"]  # [128, 2, 2048]
        for ob in range(2):
            out[b, 128 * ob : 128 * (ob + 1),
                ROWS * half : ROWS * (half + 1), :] = \
                y[:, ob, :].reshape(128, ROWS, W)
    return out



# revision 24
# speedup vs baseline: 1.6172x; 1.6172x over previous
"""Trainium2 Bass kernel for nn_MDSFF (deformable-sampling sparse attention).

Math restructuring (same algebra as the fp32r baseline, retuned for engine
balance):
  - Offsets are tanh-bounded to +-1 px, so bilinear grid-sample == 9-tap
    local stencil with per-pixel weights; w_{+1} = relu(t), w_{-1} =
    relu(-t), w_0 = 1 - |t| after clipping t only at image-edge rows/cols.
  - 1x1 convs commute with spatial shifts, so the [B,K,C,H,W] sampled
    tensor is never materialized: sim and the output combine use shifted
    views of A = k_w @ x_aux and Ao = out_w @ x_aux.
  - sim[(k,h)] = sum_t WW_t[(k)] * S_t[(h)], S_t = per-head sum of
    q * shift_t(A); G_t = sum_k WW_t*wk via a 0.25-matmul that also
    broadcasts to 128 partitions; final = sum_t G_t * shift_t(Ao).

Performance structure (vs the fp32r baseline):
  - bf16 everywhere on SBUF: DVE tensor ops hit the 2x 2-byte mode, DMA
    bytes halve, matmuls run 1 cyc/col even for <256-col streams.
  - The 3x3 offset conv runs in fp8e4m3 with DoubleRow perf mode (0.5
    cyc/col, both 128-ch blocks contracted in one matmul) and its lhsT is
    widened to 32 cols so tanh lands directly in the (k,h)-replicated T2
    layout (no separate selector matmul stage).
  - Edge clips/masks are O(rows) ops on border rows/cols only; y-bounds
    come from per-core scalars so one program serves all 8 cores.
  - Work is spread across DVE/Act/Pool via per-op assignment tables.

Sharding: 8 cores = 4 batches x 2 H-halves, 2 column chunks of 16 rows.
Host (numpy) does only data movement and dtype conversion.
"""

import sys

sys.path.insert(0, "/opt/trn_rl_repo")

import numpy as np
import ml_dtypes

import concourse.bass as bass
import concourse.mybir as mybir
from concourse import tile
from concourse.bass_utils import run_bass_kernel_spmd

# ---------------- problem constants (hardcoded per contract) ----------------
B, C, H, W = 4, 256, 64, 64
K = 8
NCORES = 8
ROWS = 32          # center rows per core
CHR = 16           # rows per chunk
N1 = CHR * W       # 1024 center pixels per chunk
HR = 34            # haloed rows per core
XA_W = HR * W      # 2176
XM_W = 2 + 66 * HR  # 2246->2248 padded x_main width: col = 2 + 66*r + w
XM_W = 2248
AW = 2 + 18 * W    # 1154: per-chunk A/Ao width, data cols [1, 1153)
TAPS = [(dy, dx) for dy in (-1, 0, 1) for dx in (-1, 0, 1)]

F32 = mybir.dt.float32
BF16 = mybir.dt.bfloat16
F8 = mybir.dt.float8e4
AF = mybir.ActivationFunctionType
OP = mybir.AluOpType
DR = mybir.MatmulPerfMode.DoubleRow

# mcat column map
MC_I32, MC_HS, MC_AVG4, MC_QB = 0, 32, 64, 96
MC_OFFB = 224      # [16, 1] tanh bias (conv channels, x/y interleaved-perm)
MC_SEL8 = 226      # [16, 64] off->T2 replication selector (x cols 0-31)
MC_W = 290
CSC = 32.0         # conv weight pre-scale (fp8 residual path)
# wcat column map
WC_QW, WC_KW, WC_OW, WC_SEL, WC_I128 = 0, 512, 1024, 1536, 1600
WC_W = 1728

# ------------- engine assignment tables (perf-tuning knobs) ---------------
# M / Fv taps routed to Pool (rest DVE); evac engines per stage.
M_POOL = {1, 3, 5, 7}
FV_POOL = {1, 4, 7}
S_EVAC = ["A"] * 9            # per tap: A=Act, V=DVE copy
GB_EVAC = ["A", "V", "A", "A", "V", "A", "A", "V", "A"]
AO_EVAC = ["A", "A", "V"] * 4   # per (dst, ob, j)
Q_EVAC = ["A"] * 4
WW_POOL = set()
QT_POOL = {2, 6}

_CACHE = {}


# ============================ program builder ===============================

def _build_program():
    MAX_WAITS = 1

    SPLIT_OK = {
        "InstDrain", "InstNoOp", "InstMatmult", "InstLdweights",
        "InstTensorTensor", "InstActivation", "InstTensorScalarPtr",
        "InstTensorReduce", "InstCopy", "InstMemSet", "InstMemset",
        "InstReciprocal", "InstTensorTensorReduce", "InstTensorCopy",
    }

    def split_waits(nc):
        # walrus in this container rejects instructions carrying more than
        # MAX_WAITS semaphore waits; spill extras onto same-engine nops.
        # Only safe for engine-FIFO instructions: hoisting a DMA descriptor's
        # wait onto the SP sequencer can deadlock.
        f = nc.m.functions[0]
        for bb in f.blocks:
            insts = bb.instructions
            out = []
            changed = False
            for inst in insts:
                si = inst.sync_info
                waits = list(si.on_wait) if si and si.on_wait else []
                if (len(waits) > MAX_WAITS
                        and type(inst).__name__ in SPLIT_OK
                        and all(w.wait_reg is None for w in waits)):
                    changed = True
                    rest, keep = waits[:-MAX_WAITS], waits[-MAX_WAITS:]
                    for i in range(0, len(rest), MAX_WAITS):
                        nop = mybir.InstNoOp(
                            name=f"Wspill_{inst.name}_{i}", ins=[], outs=[])
                        nop.engine = inst.engine
                        nop.sync_info = mybir.SyncInfo(
                            on_wait=rest[i : i + MAX_WAITS], on_update=[])
                        nc.register_instruction(nop)
                        out.append(nop)
                    inst.sync_info = mybir.SyncInfo(
                        on_wait=keep, on_update=list(si.on_update or [])
                    )
                out.append(inst)
            if changed:
                bb.instructions = out

    nc = bass.Bass("TRN2", target_bir_lowering=False, debug=False,
                   num_devices=NCORES)

    dp = nc.dram_tensor
    xq_d = dp("xq", [128, 2, 2, XM_W], F8, kind="ExternalInput")
    xm16_d = dp("xm16", [128, 2, ROWS * W], BF16, kind="ExternalInput")
    xa_d = dp("xa", [128, 2, XA_W], BF16, kind="ExternalInput")
    cwp_d = dp("cwp", [128, 9, 2, 2, 16], F8, kind="ExternalInput")
    cwr_d = dp("cwr", [128, 9, 2, 16], F8, kind="ExternalInput")
    wcat_d = dp("wcat", [128, WC_W], BF16, kind="ExternalInput")
    mcat_d = dp("mcat", [32, MC_W], BF16, kind="ExternalInput")
    ycl_d = dp("ycl", [32, 4], F32, kind="ExternalInput")
    y_d = dp("y", [128, 2, 2, N1], BF16, kind="ExternalOutput")

    V = nc.vector
    A_ = nc.scalar
    P_ = nc.gpsimd

    def mm(out, lhsT, rhs, start, stop, perf_mode=None):
        nc.tensor.matmul(out=out, lhsT=lhsT, rhs=rhs, start=start, stop=stop,
                         perf_mode=perf_mode, skip_group_check=True)

    def evac(eng, out, in_):
        if eng == "A":
            A_.activation(out=out, in_=in_, func=AF.Copy)
        elif eng == "V":
            V.tensor_copy(out=out, in_=in_)
        else:
            P_.tensor_copy(out=out, in_=in_)

    def tt(eng, out, in0, in1, op=OP.mult):
        (P_ if eng == "P" else V).tensor_tensor(out=out, in0=in0, in1=in1,
                                                op=op)

    with tile.TileContext(nc) as tc:
        with (
            nc.allow_low_precision(reason="bf16 pipeline: rounding is within "
                                   "this kernel's error budget"),
            tc.tile_pool(name="pw", bufs=1) as pw,       # weights/consts
            tc.tile_pool(name="pio", bufs=1) as pio,     # inputs
            tc.tile_pool(name="pbig", bufs=1) as pbig,   # q/A/Ao/M/Gb/Fv
            tc.tile_pool(name="pmap", bufs=1) as pmap,   # 32-row maps
            tc.tile_pool(name="psp", bufs=1, space="PSUM") as psp,
        ):
            xq = pio.tile([128, 2, 2, XM_W], F8, tag="xq")
            xm16 = pio.tile([128, 2, ROWS * W], BF16, tag="xm16")
            xa = pio.tile([128, 2, XA_W], BF16, tag="xa")
            cwp = pw.tile([128, 9, 2, 2, 16], F8, tag="cwp")
            cwr = pw.tile([128, 9, 2, 16], F8, tag="cwr")
            wcat = pw.tile([128, WC_W], BF16, tag="wcat")
            mcat = pw.tile([32, MC_W], BF16, tag="mcat")
            ycl = pw.tile([32, 4], F32, tag="ycl")
            nc.sync.dma_start(out=ycl[:], in_=ycl_d[:])
            nc.sync.dma_start(out=xq[:], in_=xq_d[:])
            nc.sync.dma_start(out=cwp[:], in_=cwp_d[:])
            nc.sync.dma_start(out=cwr[:], in_=cwr_d[:])
            nc.sync.dma_start(out=mcat[:], in_=mcat_d[:])
            nc.sync.dma_start(out=wcat[:], in_=wcat_d[:])
            nc.sync.dma_start(out=xm16[:], in_=xm16_d[:])
            for cb in range(2):
                nc.sync.dma_start(out=xa[:, cb, :], in_=xa_d[:, cb, :])

            def w4(o):  # [128, 2, 2, 128] block at col o
                return wcat[:, o : o + 512].rearrange(
                    "p (cb ob m) -> p cb ob m", cb=2, ob=2)

            qwT, kwT, owT = w4(WC_QW), w4(WC_KW), w4(WC_OW)
            sel = wcat[:, WC_SEL : WC_SEL + 64].rearrange(
                "p (cb j) -> p cb j", cb=2)
            i128 = wcat[:, WC_I128 : WC_I128 + 128]
            i32 = mcat[:, MC_I32 : MC_I32 + 32]
            hs = mcat[:, MC_HS : MC_HS + 32]
            avg4 = mcat[:, MC_AVG4 : MC_AVG4 + 32]
            qb = mcat[:, MC_QB : MC_QB + 128]
            offb = mcat[0:16, MC_OFFB : MC_OFFB + 1]
            yb = ycl

            def xq_view(ch, g, dy, dx, cb=None):
                # [128, 2, 264] fp8 view: 4 haloed rows (66-col padded,
                # contiguous) at local row (1 + 16*ch + 4*g + dy), shift dx.
                # cb=None: ktile dim = cb over the x8 plane (w-resid pass);
                # else: ktile dim = (x8, xr) of channel block cb.
                o = 2 + 66 * (1 + 16 * ch + 4 * g + dy) + dx
                if cb is None:
                    return xq[:, :, 0, o : o + 264]
                return xq[:, cb, :, o : o + 264]

            # per-chunk tiles
            q_sb, A_sb, Ao_sb, T2, w1, wm1, w0 = {}, {}, {}, {}, {}, {}, {}
            WWs, E_sb, Ff_sb, WK_sb = {}, {}, {}, {}

            # ---------------- fronts ----------------
            for ch in range(2):
                # offset conv: fp8 DoubleRow, scaled-residual 3-pass
                # (w8@x8 + w8@xr ktile-packed per cb, then wr@x8 cb-packed);
                # tanh(in/CSC + b) evacuates to off, T2 built by replicating
                # DMA (partitions (k,h) <- conv channel k).
                off = pmap.tile([16, N1], BF16, tag="off", bufs=2,
                                name=f"off{ch}")
                T2[ch] = pmap.tile([32, 2, N1], BF16, tag="T2", bufs=2,
                                   name=f"T2{ch}")
                for g in range(4):
                    cps = psp.tile([128, 512], F32, tag="pA", bufs=2,
                                   name=f"cps{ch}{g}")
                    for t in range(9):
                        dy, dx = TAPS[t]
                        for cb in range(2):
                            mm(cps[0:16, 0:264], cwp[:, t, cb, :, :],
                               xq_view(ch, g, dy, dx, cb),
                               start=(t == 0 and cb == 0), stop=False,
                               perf_mode=DR)
                        mm(cps[0:16, 0:264], cwr[:, t, :, :],
                           xq_view(ch, g, dy, dx),
                           start=False, stop=(t == 8), perf_mode=DR)
                    cin = (cps[0:16, 0:264]
                           .rearrange("p (r w) -> p r w", w=66)[:, :, 0:64])
                    tout = (off[:, 256 * g : 256 * (g + 1)]
                            .rearrange("p (r w) -> p r w", w=64))
                    A_.activation(out=tout, in_=cin, func=AF.Tanh,
                                  bias=offb, scale=1.0 / CSC)
                for j in range(2):
                    tps = psp.tile([128, 512], F32, tag="pA", bufs=2,
                                   name=f"tps{ch}{j}")
                    mm(tps[0:64, :],
                       mcat[0:16, MC_SEL8 : MC_SEL8 + 64],
                       off[:, 512 * j : 512 * (j + 1)],
                       start=True, stop=True)
                    for xy in range(2):
                        A_.activation(
                            out=T2[ch][:, xy, 512 * j : 512 * (j + 1)],
                            in_=tps[32 * xy : 32 * xy + 32, :], func=AF.Copy)

                # q projection (bf16)
                q_sb[ch] = pbig.tile([128, 2, N1], BF16, tag="q", bufs=2,
                                     name=f"q{ch}")
                for ob in range(2):
                    for i in range(2):
                        qps = psp.tile([128, 512], F32, tag="pA", bufs=2,
                                       name=f"qps{ch}{ob}{i}")
                        rhs_c = 512 * (2 * ch + i)
                        for cb in range(2):
                            mm(qps[:], qwT[:, cb, ob, :],
                               xm16[:, cb, rhs_c : rhs_c + 512],
                               start=(cb == 0), stop=(cb == 1))
                        evac(Q_EVAC[2 * ob + i],
                             q_sb[ch][:, ob, 512 * i : 512 * (i + 1)],
                             qps[:])

                # A / Ao projections (bf16)
                A_sb[ch] = pbig.tile([128, 2, AW], BF16, tag="A", bufs=2,
                                     name=f"A{ch}")
                Ao_sb[ch] = pbig.tile([128, 2, AW], BF16, tag="Ao", bufs=2,
                                      name=f"Ao{ch}")
                for dst in (A_sb[ch], Ao_sb[ch]):
                    V.memset(dst[:, :, 0:1], 0.0)
                    V.memset(dst[:, :, AW - 1 : AW], 0.0)
                ei = 0
                for di, (dst, wT) in enumerate(((A_sb[ch], kwT),
                                                (Ao_sb[ch], owT))):
                    for ob in range(2):
                        for j, sz in ((0, 512), (1, 512), (2, 128)):
                            aps = psp.tile([128, 512], F32, tag="pA", bufs=2,
                                           name=f"aps{ch}{di}{ob}{j}")
                            rc = 64 * CHR * ch + 512 * j
                            for cb in range(2):
                                mm(aps[:, 0:sz], wT[:, cb, ob, :],
                                   xa[:, cb, rc : rc + sz],
                                   start=(cb == 0), stop=(cb == 1))
                            evac(AO_EVAC[ei],
                                 dst[:, ob, 1 + 512 * j : 1 + 512 * j + sz],
                                 aps[:, 0:sz])
                            ei += 1

                # ------------- tap-weight maps (DVE, mostly 4x) -------------
                t2 = T2[ch]
                t2x = t2[:, 0, :].rearrange("p (r w) -> p r w", w=64)
                # edge clips: x at cols 0/63 (consts), y at first/last row
                # (per-core scalars; inert bounds elsewhere)
                V.tensor_scalar_max(out=t2x[:, :, 0:1], in0=t2x[:, :, 0:1],
                                    scalar1=-0.5)
                V.tensor_scalar_min(out=t2x[:, :, 63:64],
                                    in0=t2x[:, :, 63:64], scalar1=0.5)
                V.tensor_scalar_max(out=t2[:, 1, 0:64], in0=t2[:, 1, 0:64],
                                    scalar1=yb[:, 2 * ch : 2 * ch + 1])
                V.tensor_scalar_min(out=t2[:, 1, N1 - 64 : N1],
                                    in0=t2[:, 1, N1 - 64 : N1],
                                    scalar1=yb[:, 2 * ch + 1 : 2 * ch + 2])
                w1[ch] = pmap.tile([32, 2, N1], BF16, tag="w1", bufs=2,
                                   name=f"w1_{ch}")
                wm1[ch] = pmap.tile([32, 2, N1], BF16, tag="wm1", bufs=2,
                                    name=f"wm1_{ch}")
                w0[ch] = pmap.tile([32, 2, N1], BF16, tag="w0", bufs=2,
                                   name=f"w0_{ch}")
                V.tensor_scalar_max(out=w1[ch][:], in0=t2[:], scalar1=0.0)
                V.tensor_scalar(out=wm1[ch][:], in0=t2[:], scalar1=-1.0,
                                scalar2=0.0, op0=OP.mult, op1=OP.max)
                V.tensor_tensor(out=w0[ch][:], in0=w1[ch][:], in1=wm1[ch][:],
                                op=OP.add)
                V.tensor_scalar(out=w0[ch][:], in0=w0[ch][:], scalar1=-1.0,
                                scalar2=1.0, op0=OP.mult, op1=OP.add)
                # x edge masks: left tap dead at col 0, right tap at col 63
                wm1x = wm1[ch][:, 0, :].rearrange("p (r w) -> p r w", w=64)
                w1x = w1[ch][:, 0, :].rearrange("p (r w) -> p r w", w=64)
                V.memset(wm1x[:, :, 0:1], 0.0)
                V.memset(w1x[:, :, 63:64], 0.0)

                WX = {-1: wm1[ch][:, 0, :], 0: w0[ch][:, 0, :],
                      1: w1[ch][:, 0, :]}
                WY = {-1: wm1[ch][:, 1, :], 0: w0[ch][:, 1, :],
                      1: w1[ch][:, 1, :]}
                WWs[ch] = []
                for t in range(9):
                    dy, dx = TAPS[t]
                    ww = pmap.tile([32, N1], BF16, tag="WW", bufs=18,
                                   name=f"WW{ch}{t}")
                    tt("P" if t in WW_POOL else "V", ww[:], WY[dy][:],
                       WX[dx][:])
                    WWs[ch].append(ww)

            # ---------------- per-chunk S loop / softmax / combine ----------
            for ch in range(2):
                sim_ps = psp.tile([32, N1], F32, tag="psim", bufs=1,
                                  name=f"sim{ch}")
                for t in range(9):
                    dy, dx = TAPS[t]
                    o_t = 65 + 64 * dy + dx
                    M = pbig.tile([128, 2, N1], BF16, tag="M", bufs=2,
                                  name=f"M{ch}{t}")
                    tt("P" if t in M_POOL else "V", M[:], q_sb[ch][:],
                       A_sb[ch][:, :, o_t : o_t + N1])
                    sps = psp.tile([32, N1], F32, tag="psS", bufs=1,
                                   name=f"sps{ch}{t}")
                    for j in range(2):
                        for cb in range(2):
                            mm(sps[:, 512 * j : 512 * (j + 1)], sel[:, cb, :],
                               M[:, cb, 512 * j : 512 * (j + 1)],
                               start=(cb == 0), stop=(cb == 1))
                    S_t = pmap.tile([32, N1], BF16, tag="S", bufs=3,
                                    name=f"S{ch}{t}")
                    evac(S_EVAC[t], S_t[:], sps[:])
                    P_t = pmap.tile([32, N1], BF16, tag="P", bufs=4,
                                    name=f"P{ch}{t}")
                    V.tensor_tensor(out=P_t[:], in0=WWs[ch][t][:], in1=S_t[:],
                                    op=OP.mult)
                    for j in range(2):
                        sl = slice(512 * j, 512 * (j + 1))
                        mm(sim_ps[:, sl], i32, P_t[:, sl],
                           start=(t == 0), stop=(t == 8))

                # softmax over K (per head), then head-average -> WK
                E_sb[ch] = pmap.tile([32, N1], BF16, tag="E", bufs=2,
                                     name=f"E{ch}")
                A_.activation(out=E_sb[ch][:], in_=sim_ps[:], func=AF.Exp,
                              scale=0.125)
                Ff_sb[ch] = pmap.tile([32, N1], BF16, tag="Ff", bufs=2,
                                      name=f"Ff{ch}")
                for j in range(2):
                    sl = slice(512 * j, 512 * (j + 1))
                    dps = psp.tile([128, 512], F32, tag="pA", bufs=2,
                                   name=f"dps{ch}{j}")
                    mm(dps[0:32, :], hs, E_sb[ch][:, sl], start=True,
                       stop=True)
                    R_t = pmap.tile([32, 512], BF16, tag="R", bufs=2,
                                    name=f"R{ch}{j}")
                    V.reciprocal(out=R_t[:], in_=dps[0:32, :])
                    V.tensor_tensor(out=Ff_sb[ch][:, sl],
                                    in0=E_sb[ch][:, sl], in1=R_t[:],
                                    op=OP.mult)
                WK_sb[ch] = pmap.tile([32, N1], BF16, tag="WK", bufs=2,
                                      name=f"WK{ch}")
                for j in range(2):
                    sl = slice(512 * j, 512 * (j + 1))
                    wps = psp.tile([128, 512], F32, tag="pA", bufs=2,
                                   name=f"wps{ch}{j}")
                    mm(wps[0:32, :], avg4, Ff_sb[ch][:, sl], start=True,
                       stop=True)
                    A_.activation(out=WK_sb[ch][:, sl], in_=wps[0:32, :],
                                  func=AF.Copy)

                # G stage A: per-tap pixel gains broadcast to 128 partitions
                Gbs = []
                for t in range(9):
                    Q_t = pmap.tile([32, N1], BF16, tag="Qt", bufs=4,
                                    name=f"Q{ch}{t}")
                    tt("P" if t in QT_POOL else "V", Q_t[:], WWs[ch][t][:],
                       WK_sb[ch][:])
                    Gb = pbig.tile([128, N1], BF16, tag="Gb", bufs=10,
                                   name=f"Gb{ch}{t}")
                    for j in range(2):
                        sl = slice(512 * j, 512 * (j + 1))
                        gps = psp.tile([128, 512], F32, tag="pA", bufs=2,
                                       name=f"gps{ch}{t}{j}")
                        mm(gps[:], qb, Q_t[:, sl], start=True, stop=True)
                        evac(GB_EVAC[t], Gb[:, sl], gps[:])
                    Gbs.append(Gb)

                # G stage B: two output-channel passes, PE-accumulated
                for ob in range(2):
                    fin = psp.tile([128, N1], F32, tag="pfin", bufs=1,
                                   name=f"fin{ch}{ob}")
                    for t in range(9):
                        dy, dx = TAPS[t]
                        o_t = 65 + 64 * dy + dx
                        Fv = pbig.tile([128, N1], BF16, tag="Fv", bufs=3,
                                       name=f"Fv{ch}{t}{ob}")
                        tt("P" if t in FV_POOL else "V", Fv[:], Gbs[t][:],
                           Ao_sb[ch][:, ob, o_t : o_t + N1])
                        for j in range(2):
                            sl = slice(512 * j, 512 * (j + 1))
                            mm(fin[:, sl], i128, Fv[:, sl],
                               start=(t == 0), stop=(t == 8))
                    osb = pbig.tile([128, N1], BF16, tag="osb", bufs=2,
                                    name=f"osb{ch}{ob}")
                    A_.activation(out=osb[:], in_=fin[:], func=AF.Copy)
                    nc.gpsimd.dma_start(out=y_d[:, ch, ob, :], in_=osb[:])

    split_waits(nc)
    return nc


# ============================ host-side prep ===============================

def _consts():
    i32 = np.eye(32, dtype=np.float32)
    hs = np.zeros((32, 32), np.float32)
    avg4 = np.zeros((32, 32), np.float32)
    for i in range(32):
        for j in range(32):
            if i % 4 == j % 4:
                hs[i, j] = 1.0
            if i // 4 == j // 4:
                avg4[i, j] = 0.25
    qb = np.full((32, 128), 0.25, np.float32)
    i128 = np.eye(128, dtype=np.float32)
    sel = np.zeros((128, 2, 32), np.float32)
    for cb in range(2):
        for p in range(128):
            h = (128 * cb + p) // 64
            for j in range(32):
                if j % 4 == h:
                    sel[p, cb, j] = 1.0
    return i32, hs, avg4, qb, i128, sel


def _prep_inputs(x_main, x_aux, offset_w, offset_b, q_w, k_w, out_w):
    i32, hs, avg4, qb, i128, sel = _consts()
    bf16 = ml_dtypes.bfloat16
    f8 = ml_dtypes.float8_e4m3

    def wT(wmat):
        # [128, 2, 2, 128]: lhsT[cin_local, cb, ob, o_local] = w[o, cin]
        r = np.zeros((128, 2, 2, 128), np.float32)
        for cb in range(2):
            for ob in range(2):
                r[:, cb, ob, :] = wmat[128 * ob : 128 * (ob + 1),
                                       128 * cb : 128 * (cb + 1)].T
        return r

    # conv weights: fp8 scaled-residual pair; perm puts x-offset channels at
    # rows 0-7 of off, y at 8-15
    perm = [2 * k for k in range(K)] + [2 * k + 1 for k in range(K)]
    cw = np.zeros((128, 9, 2, 16), np.float32)
    for t, (dy, dx) in enumerate(TAPS):
        for cb in range(2):
            cw[:, t, cb, :] = (CSC * offset_w[perm, 128 * cb : 128 * (cb + 1),
                                              dy + 1, dx + 1]).T
    cw8 = cw.astype(f8).astype(np.float32)
    cwr = (cw - cw8).astype(f8)
    cwp = np.repeat(cw8[:, :, :, None, :], 2, axis=3).astype(f8)

    wcat = np.zeros((128, WC_W), np.float32)
    wcat[:, WC_QW : WC_QW + 512] = wT(q_w).reshape(128, 512)
    wcat[:, WC_KW : WC_KW + 512] = wT(k_w).reshape(128, 512)
    wcat[:, WC_OW : WC_OW + 512] = wT(out_w).reshape(128, 512)
    wcat[:, WC_SEL : WC_SEL + 64] = sel.reshape(128, 64)
    wcat[:, WC_I128 : WC_I128 + 128] = i128

    mcat0 = np.zeros((32, MC_W), np.float32)
    mcat0[:, MC_I32 : MC_I32 + 32] = i32
    mcat0[:, MC_HS : MC_HS + 32] = hs
    mcat0[:, MC_AVG4 : MC_AVG4 + 32] = avg4
    mcat0[:, MC_QB : MC_QB + 128] = qb
    mcat0[0:16, MC_OFFB] = offset_b[perm]
    for jj in range(32):
        mcat0[jj // 4, MC_SEL8 + jj] = 1.0           # tx: off row k
        mcat0[8 + jj // 4, MC_SEL8 + 32 + jj] = 1.0  # ty: off row 8+k

    in_maps = []
    for core in range(NCORES):
        b, half = core // 2, core % 2
        h0 = ROWS * half
        xm = np.zeros((128, 2, XM_W), np.float32)
        xa = np.zeros((128, 2, XA_W), np.float32)
        for r in range(HR):
            g = h0 - 1 + r
            if 0 <= g < H:
                for cb in range(2):
                    xm[:, cb, 2 + 66 * r : 2 + 66 * r + 64] = \
                        x_main[b, 128 * cb : 128 * (cb + 1), g, :]
                    xa[:, cb, 64 * r : 64 * r + 64] = \
                        x_aux[b, 128 * cb : 128 * (cb + 1), g, :]
        xm16 = np.zeros((128, 2, ROWS * W), np.float32)
        for cb in range(2):
            xm16[:, cb, :] = x_main[b, 128 * cb : 128 * (cb + 1),
                                    h0 : h0 + ROWS, :].reshape(128, -1)
        ycl = np.zeros((32, 4), np.float32)
        for ch in range(2):
            top, bot = h0 + CHR * ch, h0 + CHR * ch + CHR - 1
            ycl[:, 2 * ch] = -0.5 if top == 0 else -4.0
            ycl[:, 2 * ch + 1] = 0.5 if bot == H - 1 else 4.0
        x8 = xm.astype(f8).astype(np.float32)
        xq = np.stack([x8, xm - x8], axis=2).astype(f8)  # [128, 2, 2, XM_W]
        in_maps.append(dict(
            xq=xq, xm16=xm16.astype(bf16), xa=xa.astype(bf16),
            cwp=cwp, cwr=cwr, wcat=wcat.astype(bf16),
            mcat=mcat0.astype(bf16), ycl=ycl))
    return in_maps


def kernel(**inputs):
    inputs = {k: np.asarray(v, dtype=np.float32) for k, v in inputs.items()}
    if "nc" not in _CACHE:
        _CACHE["nc"] = _build_program()
    nc = _CACHE["nc"]
    in_maps = _prep_inputs(
        inputs["x_main"], inputs["x_aux"], inputs["offset_w"],
        inputs["offset_b"], inputs["q_w"], inputs["k_w"], inputs["out_w"])
    res = run_bass_kernel_spmd(nc, in_maps, list(range(NCORES))).results

    out = np.zeros((B, C, H, W), np.float32)
    for core in range(NCORES):
        b, half = core // 2, core % 2
        y = np.asarray(res[core]["y"]).astype(np.float32)  # [128, 2, 2, N1]
        for ch in range(2):
            for ob in range(2):
                out[b, 128 * ob : 128 * (ob + 1),
                    ROWS * half + CHR * ch : ROWS * half + CHR * (ch + 1),
                    :] = y[:, ch, ob, :].reshape(128, CHR, W)
    return out


# revision 84
# speedup vs baseline: 1.9717x; 1.2193x over previous
"""Trainium2 Bass kernel for nn_MDSFF (deformable-sampling sparse attention).

Math restructuring (same algebra as the fp32r baseline, retuned for engine
balance):
  - Offsets are tanh-bounded to +-1 px, so bilinear grid-sample == 9-tap
    local stencil with per-pixel weights; w_{+1} = relu(t), w_{-1} =
    relu(-t), w_0 = 1 - |t| after clipping t only at image-edge rows/cols.
  - 1x1 convs commute with spatial shifts, so the [B,K,C,H,W] sampled
    tensor is never materialized: sim and the output combine use shifted
    views of A = k_w @ x_aux and Ao = out_w @ x_aux.
  - sim[(k,h)] = sum_t WW_t[(k)] * S_t[(h)], S_t = per-head sum of
    q * shift_t(A); G_t = sum_k WW_t*wk via a 0.25-matmul that also
    broadcasts to 128 partitions; final = sum_t G_t * shift_t(Ao).

Performance structure (vs the fp32r baseline):
  - bf16 everywhere on SBUF: DVE tensor ops hit the 2x 2-byte mode, DMA
    bytes halve, matmuls run 1 cyc/col even for <256-col streams.
  - The 3x3 offset conv runs in fp8e4m3 with DoubleRow perf mode (0.5
    cyc/col, 2 contraction rows per partition) using a scaled-residual
    3-pass scheme -- w8@x8 + w8@xr + wr@x8 with weights pre-scaled by CSC
    to dodge fp8 denormals -- which is both cheaper than bf16 (1.5 vs 2
    cyc/col) and more accurate (~0.14% rms).
  - The 3 dx-taps of each dy row are computed in single wide DVE ops via
    overlapping stride-1 access patterns (A/Ao shifted views differ by one
    column), cutting op counts and sync hops 3x for the map algebra.
  - Edge clips/masks are O(rows) ops on border rows/cols only; y-bounds
    come from per-core scalars so one program serves all 8 cores.
  - Work is spread across DVE/Act/Pool via per-op assignment tables
    (gpsimd cannot touch PSUM, so PSUM evacuations stay on Act/DVE).
  - Emission order software-pipelines the two chunks (chunk-0 S-loop
    outranks chunk-1's conv front; G stages interleave with the next
    chunk's S loop) under the readiness-based tile scheduler.

Sharding: 8 cores = 4 batches x 2 H-halves, 2 column chunks of 16 rows.
Host (numpy) does only data movement and dtype conversion.
"""

import sys

sys.path.insert(0, "/opt/trn_rl_repo")

import numpy as np
import ml_dtypes

import concourse.bass as bass
import concourse.mybir as mybir
from concourse import tile
from concourse.bass_utils import run_bass_kernel_spmd

# ---------------- problem constants (hardcoded per contract) ----------------
B, C, H, W = 4, 256, 64, 64
K = 8
NCORES = 8
ROWS = 32          # center rows per core
CHR = 16           # rows per chunk
N1 = CHR * W       # 1024 center pixels per chunk
HR = 34            # haloed rows per core
XA_W = HR * W      # 2176
XM_W = 2 + 66 * HR  # 2246->2248 padded x_main width: col = 2 + 66*r + w
XM_W = 2248
AW = 2 + 18 * W    # 1154: per-chunk A/Ao width, data cols [1, 1153)
TAPS = [(dy, dx) for dy in (-1, 0, 1) for dx in (-1, 0, 1)]

F32 = mybir.dt.float32
BF16 = mybir.dt.bfloat16
F8 = mybir.dt.float8e4
AF = mybir.ActivationFunctionType
OP = mybir.AluOpType
DR = mybir.MatmulPerfMode.DoubleRow

# mcat column map
MC_I32, MC_HS, MC_AVG4, MC_QB = 0, 32, 64, 96
MC_OFFB = 224      # [16, 1] tanh bias (conv channels, x/y interleaved-perm)
MC_SEL8 = 226      # [16, 64] off->T2 replication selector (x cols 0-31)
MC_W = 290
CSC = 32.0         # conv weight pre-scale (fp8 residual path)
# wcat column map
WC_QW, WC_KW, WC_OW, WC_SEL, WC_I128 = 0, 512, 1024, 1536, 1600
WC_W = 1728

# ------------- engine assignment tables (perf-tuning knobs) ---------------
# M / Fv taps routed to Pool (rest DVE); evac engines per stage
# (A=Act, V=DVE, P=Pool).
M_POOL = {3, 4}
FV_POOL = {(0, 0, 1), (0, 1, 1), (1, 0, 1)}
S_EVAC = ["A"] * 9
S_EVAC_J1 = ["A", "V", "A", "A", "V", "A", "A", "V", "A"]
GB_EVAC = ["A", "A", "A", "A", "V", "A", "A", "A", "A"]
AO_EVAC = ["A", "A", "V"] * 4
Q_EVAC = ["A"] * 4
WW_POOL = {1, 2}
QT_POOL = {(0, 1), (1, 1)}

# packed const blob (bf16 columns): [wcat+A-weights | small consts]
WB_CWP = WC_W          # 288 bf16 cols = 576 fp8
WB_CWR = WB_CWP + 288  # 144 bf16 cols = 288 fp8
WB_YCL = WB_CWR + 144  # 8 bf16 cols = 4 fp32 (rows 0:32)
WB_MC = WB_YCL + 8     # mcat [32, MC_W]
WB_W = WB_MC + MC_W

_CACHE = {}


# ============================ program builder ===============================

def _build_program():
    MAX_WAITS = 1

    SPLIT_OK = {
        "InstDrain", "InstNoOp", "InstMatmult", "InstLdweights",
        "InstTensorTensor", "InstActivation", "InstTensorScalarPtr",
        "InstTensorReduce", "InstCopy", "InstMemSet", "InstMemset",
        "InstReciprocal", "InstTensorTensorReduce", "InstTensorCopy",
    }

    def split_waits(nc):
        # walrus in this container rejects instructions carrying more than
        # MAX_WAITS semaphore waits; spill extras onto same-engine nops.
        # Only safe for engine-FIFO instructions: hoisting a DMA descriptor's
        # wait onto the SP sequencer can deadlock.
        f = nc.m.functions[0]
        for bb in f.blocks:
            insts = bb.instructions
            out = []
            changed = False
            for inst in insts:
                si = inst.sync_info
                waits = list(si.on_wait) if si and si.on_wait else []
                if (len(waits) > MAX_WAITS
                        and type(inst).__name__ in SPLIT_OK
                        and all(w.wait_reg is None for w in waits)):
                    changed = True
                    rest, keep = waits[:-MAX_WAITS], waits[-MAX_WAITS:]
                    for i in range(0, len(rest), MAX_WAITS):
                        nop = mybir.InstNoOp(
                            name=f"Wspill_{inst.name}_{i}", ins=[], outs=[])
                        nop.engine = inst.engine
                        nop.sync_info = mybir.SyncInfo(
                            on_wait=rest[i : i + MAX_WAITS], on_update=[])
                        nc.register_instruction(nop)
                        out.append(nop)
                    inst.sync_info = mybir.SyncInfo(
                        on_wait=keep, on_update=list(si.on_update or [])
                    )
                out.append(inst)
            if changed:
                bb.instructions = out

    nc = bass.Bass("TRN2", target_bir_lowering=False, debug=False,
                   num_devices=NCORES)

    dp = nc.dram_tensor
    xq_d = dp("xq", [128, 2, 2, XM_W], F8, kind="ExternalInput")
    xm16_d = dp("xm16", [128, 2, ROWS * W], BF16, kind="ExternalInput")
    xa_d = dp("xa", [128, 2, XA_W], BF16, kind="ExternalInput")
    wcons_d = dp("wcons", [128, WB_W], BF16, kind="ExternalInput")
    y_d = dp("y", [128, 2, 2, N1], BF16, kind="ExternalOutput")

    V = nc.vector
    A_ = nc.scalar
    P_ = nc.gpsimd

    def mm(out, lhsT, rhs, start, stop, perf_mode=None):
        nc.tensor.matmul(out=out, lhsT=lhsT, rhs=rhs, start=start, stop=stop,
                         perf_mode=perf_mode, skip_group_check=True)

    def evac(eng, out, in_):
        if eng == "A":
            A_.activation(out=out, in_=in_, func=AF.Copy)
        elif eng == "V":
            V.tensor_copy(out=out, in_=in_)
        else:
            P_.tensor_copy(out=out, in_=in_)

    def tt(eng, out, in0, in1, op=OP.mult):
        (P_ if eng == "P" else V).tensor_tensor(out=out, in0=in0, in1=in1,
                                                op=op)

    def dx3(apview, n):
        # [P, 3, n] overlapping view: dx in {-1, 0, +1} at column stride 1
        c = apview.unsqueeze(1).broadcast_to([apview.shape[0], 3, n]).copy()
        c.ap[1] = [1, 3]
        return c

    with tile.TileContext(nc) as tc:
        with (
            nc.allow_low_precision(reason="bf16 pipeline: rounding is within "
                                   "this kernel's error budget"),
            tc.tile_pool(name="pw", bufs=1) as pw,       # weights/consts
            tc.tile_pool(name="pio", bufs=1) as pio,     # inputs
            tc.tile_pool(name="pbig", bufs=1) as pbig,   # q/A/Ao/M/Gb/Fv
            tc.tile_pool(name="pmap", bufs=1) as pmap,   # 32-row maps
            tc.tile_pool(name="psp", bufs=1, space="PSUM") as psp,
        ):
            xq = pio.tile([128, 2, 2, XM_W], F8, tag="xq")
            xm16 = pio.tile([128, 2, ROWS * W], BF16, tag="xm16")
            xa = pio.tile([128, 2, XA_W], BF16, tag="xa")
            wcat_t = pw.tile([128, WC_W], BF16, tag="wcat")
            cwc = pw.tile([128, WB_W - WB_CWP], BF16, tag="cwc")
            XQA = 2 + 66 * 8   # first conv group's rows
            XQ0 = 2 + 66 * 19  # cols holding chunk-0's haloed conv rows
            nc.sync.dma_start(out=cwc[:], in_=wcons_d[:, WB_CWP:])
            nc.sync.dma_start(out=xq[:, :, :, 0:XQA], in_=xq_d[:, :, :, 0:XQA])
            nc.sync.dma_start(out=xq[:, :, :, XQA:XQ0],
                              in_=xq_d[:, :, :, XQA:XQ0])
            nc.sync.dma_start(out=wcat_t[:], in_=wcons_d[:, 0:WC_W])
            for cb in range(2):
                nc.sync.dma_start(out=xa[:, cb, :], in_=xa_d[:, cb, :])
            for cb in range(2):
                nc.sync.dma_start(out=xm16[:, cb, :], in_=xm16_d[:, cb, :])
            nc.sync.dma_start(out=xq[:, :, :, XQ0:], in_=xq_d[:, :, :, XQ0:])
            wcat = wcat_t[:, :]
            cwp = (cwc[:, 0:288].bitcast(F8)
                   .rearrange("p (t cb kt o) -> p t cb kt o", t=9, cb=2, kt=2))
            cwr = (cwc[:, 288:432].bitcast(F8)
                   .rearrange("p (t cb o) -> p t cb o", t=9, cb=2))
            mcat = cwc[0:32, WB_MC - WB_CWP : WB_MC - WB_CWP + MC_W]
            ycl = cwc[0:32, WB_YCL - WB_CWP : WB_YCL - WB_CWP + 8].bitcast(F32)

            def w4(o):  # [128, 2, 2, 128] block at col o
                return wcat[:, o : o + 512].rearrange(
                    "p (cb ob m) -> p cb ob m", cb=2, ob=2)

            qwT, kwT, owT = w4(WC_QW), w4(WC_KW), w4(WC_OW)
            sel = wcat[:, WC_SEL : WC_SEL + 64].rearrange(
                "p (cb j) -> p cb j", cb=2)
            i128 = wcat[:, WC_I128 : WC_I128 + 128]
            i32 = mcat[:, MC_I32 : MC_I32 + 32]
            hs = mcat[:, MC_HS : MC_HS + 32]
            avg4 = mcat[:, MC_AVG4 : MC_AVG4 + 32]
            qb = mcat[:, MC_QB : MC_QB + 128]
            offb = mcat[0:16, MC_OFFB : MC_OFFB + 1]
            yb = ycl

            def xq_view(ch, g, dy, dx, cb=None):
                # [128, 2, 264] fp8 view: 4 haloed rows (66-col padded,
                # contiguous) at local row (1 + 16*ch + 4*g + dy), shift dx.
                # cb=None: ktile dim = cb over the x8 plane (w-resid pass);
                # else: ktile dim = (x8, xr) of channel block cb.
                o = 2 + 66 * (1 + 16 * ch + 4 * g + dy) + dx
                if cb is None:
                    return xq[:, :, 0, o : o + 264]
                return xq[:, cb, :, o : o + 264]

            # per-chunk tiles
            q_sb, A_sb, Ao_sb, T2 = {}, {}, {}, {}
            WWs, E_sb, Ff_sb, WK_sb = {}, {}, {}, {}

            # ---------------- fronts ----------------
            for ch in range(2):
                # offset conv: fp8 DoubleRow, scaled-residual 3-pass
                # (w8@x8 + w8@xr ktile-packed per cb, then wr@x8 cb-packed);
                # tanh(in/CSC + b) evacuates to off, T2 built by replicating
                # DMA (partitions (k,h) <- conv channel k).
                off = pmap.tile([16, N1], BF16, tag="off", bufs=2,
                                name=f"off{ch}")
                T2[ch] = pmap.tile([32, 2, N1], BF16, tag="T2", bufs=2,
                                   name=f"T2{ch}")
                for g in range(4):
                    cps = psp.tile([128, 512], F32, tag="pA", bufs=2,
                                   name=f"cps{ch}{g}")
                    for t in range(9):
                        dy, dx = TAPS[t]
                        for cb in range(2):
                            mm(cps[0:16, 0:264], cwp[:, t, cb, :, :],
                               xq_view(ch, g, dy, dx, cb),
                               start=(t == 0 and cb == 0), stop=False,
                               perf_mode=DR)
                        mm(cps[0:16, 0:264], cwr[:, t, :, :],
                           xq_view(ch, g, dy, dx),
                           start=False, stop=(t == 8), perf_mode=DR)
                    cin = (cps[0:16, 0:264]
                           .rearrange("p (r w) -> p r w", w=66)[:, :, 0:64])
                    tout = (off[:, 256 * g : 256 * (g + 1)]
                            .rearrange("p (r w) -> p r w", w=64))
                    A_.activation(out=tout, in_=cin, func=AF.Tanh,
                                  bias=offb, scale=1.0 / CSC)
                    if g % 2 == 1:
                        # j-half of off complete -> replicate into T2 early
                        j = g // 2
                        tps = psp.tile([128, 512], F32, tag="pA", bufs=2,
                                       name=f"tps{ch}{j}")
                        mm(tps[0:64, :],
                           mcat[0:16, MC_SEL8 : MC_SEL8 + 64],
                           off[:, 512 * j : 512 * (j + 1)],
                           start=True, stop=True)
                        for xy in range(2):
                            A_.activation(
                                out=T2[ch][:, xy, 512 * j : 512 * (j + 1)],
                                in_=tps[32 * xy : 32 * xy + 32, :],
                                func=AF.Copy)

                # q projection (bf16)
                q_sb[ch] = pbig.tile([128, 2, N1], BF16, tag="q", bufs=2,
                                     name=f"q{ch}")
                for ob in range(2):
                    for i in range(2):
                        qps = psp.tile([128, 512], F32, tag="pA", bufs=2,
                                       name=f"qps{ch}{ob}{i}")
                        rhs_c = 512 * (2 * ch + i)
                        for cb in range(2):
                            mm(qps[:], qwT[:, cb, ob, :],
                               xm16[:, cb, rhs_c : rhs_c + 512],
                               start=(cb == 0), stop=(cb == 1))
                        evac(Q_EVAC[2 * ob + i],
                             q_sb[ch][:, ob, 512 * i : 512 * (i + 1)],
                             qps[:])

                # A / Ao projections (bf16)
                A_sb[ch] = pbig.tile([128, 2, AW], BF16, tag="A", bufs=2,
                                     name=f"A{ch}")
                Ao_sb[ch] = pbig.tile([128, 2, AW], BF16, tag="Ao", bufs=2,
                                      name=f"Ao{ch}")
                for dst in (A_sb[ch], Ao_sb[ch]):
                    V.memset(dst[:, :, 0:1], 0.0)
                    V.memset(dst[:, :, AW - 1 : AW], 0.0)
                ei = 0
                for di, (dst, wT) in enumerate(((A_sb[ch], kwT),
                                                (Ao_sb[ch], owT))):
                    for ob in range(2):
                        for j, sz in ((0, 512), (1, 512), (2, 128)):
                            aps = psp.tile([128, 512], F32, tag="pA", bufs=2,
                                           name=f"aps{ch}{di}{ob}{j}")
                            rc = 64 * CHR * ch + 512 * j
                            for cb in range(2):
                                mm(aps[:, 0:sz], wT[:, cb, ob, :],
                                   xa[:, cb, rc : rc + sz],
                                   start=(cb == 0), stop=(cb == 1))
                            evac(AO_EVAC[ei],
                                 dst[:, ob, 1 + 512 * j : 1 + 512 * j + sz],
                                 aps[:, 0:sz])
                            ei += 1

                # ------------- tap-weight maps (DVE, mostly 4x) -------------
                # W3[:, d+1, xy, :] = weight of tap offset d; j-split so DVE
                # starts as soon as each T2 half lands.
                t2 = T2[ch]
                W3 = pmap.tile([32, 3, 2, N1], BF16, tag="W3", bufs=1,
                               name=f"W3_{ch}")
                for j in range(2):
                    sl = slice(512 * j, 512 * (j + 1))
                    t2x = t2[:, 0, sl].rearrange("p (r w) -> p r w", w=64)
                    # edge clips: x at cols 0/63 (consts), y at first/last
                    # row (per-core scalars; inert bounds elsewhere)
                    V.tensor_scalar_max(out=t2x[:, :, 0:1],
                                        in0=t2x[:, :, 0:1], scalar1=-0.5)
                    V.tensor_scalar_min(out=t2x[:, :, 63:64],
                                        in0=t2x[:, :, 63:64], scalar1=0.5)
                    if j == 0:
                        V.tensor_scalar_max(
                            out=t2[:, 1, 0:64], in0=t2[:, 1, 0:64],
                            scalar1=yb[:, 2 * ch : 2 * ch + 1])
                    else:
                        V.tensor_scalar_min(
                            out=t2[:, 1, N1 - 64 : N1],
                            in0=t2[:, 1, N1 - 64 : N1],
                            scalar1=yb[:, 2 * ch + 1 : 2 * ch + 2])
                    V.tensor_scalar_max(out=W3[:, 2, :, sl], in0=t2[:, :, sl],
                                        scalar1=0.0)
                    V.tensor_scalar(out=W3[:, 0, :, sl], in0=t2[:, :, sl],
                                    scalar1=-1.0, scalar2=0.0, op0=OP.mult,
                                    op1=OP.max)
                    V.tensor_tensor(out=W3[:, 1, :, sl], in0=W3[:, 2, :, sl],
                                    in1=W3[:, 0, :, sl], op=OP.add)
                    V.tensor_scalar(out=W3[:, 1, :, sl], in0=W3[:, 1, :, sl],
                                    scalar1=-1.0, scalar2=1.0, op0=OP.mult,
                                    op1=OP.add)
                    # x edge masks: left tap dead at col 0, right at col 63
                    wm1x = W3[:, 0, 0, sl].rearrange("p (r w) -> p r w", w=64)
                    w1x = W3[:, 2, 0, sl].rearrange("p (r w) -> p r w", w=64)
                    V.memset(wm1x[:, :, 0:1], 0.0)
                    V.memset(w1x[:, :, 63:64], 0.0)

                # WW3[dy][:, dxi, :] = wy[dy] * wx[dx]
                WWs[ch] = []
                for dy in (-1, 0, 1):
                    ww = pmap.tile([32, 3, N1], BF16, tag="WW3", bufs=4,
                                   name=f"WW{ch}{dy}")
                    tt("P" if (dy + 1) in WW_POOL else "V", ww[:],
                       W3[:, dy + 1, 1, None, :].broadcast_to([32, 3, N1]),
                       W3[:, :, 0, :])
                    WWs[ch].append(ww)

            # ------- per-chunk S loop / softmax / combine (interleaved) -------
            sim_tiles, fin_state = {}, {}

            def s_dy(ch, dyi):
                if dyi == 0:
                    sim_tiles[ch] = psp.tile([128, N1], F32, tag="acc",
                                             bufs=2, name=f"sim{ch}")
                sim_ps = sim_tiles[ch][0:32, :]
                o_y = 65 + 64 * (dyi - 1)
                S3 = pmap.tile([32, 3, N1], BF16, tag="S3", bufs=2,
                               name=f"S{ch}{dyi}")
                for dxi in range(3):
                    t = 3 * dyi + dxi
                    o_t = o_y + dxi - 1
                    M = pbig.tile([128, 2, N1], BF16, tag="M", bufs=2,
                                  name=f"M{ch}{t}")
                    tt("P" if t in M_POOL else "V", M[:], q_sb[ch][:],
                       A_sb[ch][:, :, o_t : o_t + N1])
                    for j in range(2):
                        sps = psp.tile([32, 512], F32, tag="psS", bufs=2,
                                       name=f"sps{ch}{t}{j}")
                        for cb in range(2):
                            mm(sps[:], sel[:, cb, :],
                               M[:, cb, 512 * j : 512 * (j + 1)],
                               start=(cb == 0), stop=(cb == 1))
                        evac(S_EVAC[t] if j == 0 else S_EVAC_J1[t],
                             S3[:, dxi, 512 * j : 512 * (j + 1)], sps[:])
                P3 = pmap.tile([32, 3, N1], BF16, tag="P3", bufs=2,
                               name=f"P{ch}{dyi}")
                V.tensor_tensor(out=P3[:], in0=WWs[ch][dyi][:], in1=S3[:],
                                op=OP.mult)
                for dxi in range(3):
                    for j in range(2):
                        sl = slice(512 * j, 512 * (j + 1))
                        mm(sim_ps[:, sl], i32, P3[:, dxi, sl],
                           start=(dyi == 0 and dxi == 0),
                           stop=(dyi == 2 and dxi == 2))

            def softmax(ch):
                sim_ps = sim_tiles[ch][0:32, :]
                E_sb[ch] = pmap.tile([32, N1], BF16, tag="E", bufs=2,
                                     name=f"E{ch}")
                for j in range(2):
                    sl = slice(512 * j, 512 * (j + 1))
                    A_.activation(out=E_sb[ch][:, sl], in_=sim_ps[:, sl],
                                  func=AF.Exp, scale=0.125)
                Ff_sb[ch] = pmap.tile([32, N1], BF16, tag="Ff", bufs=2,
                                      name=f"Ff{ch}")
                for j in range(2):
                    sl = slice(512 * j, 512 * (j + 1))
                    dps = psp.tile([128, 512], F32, tag="pA", bufs=2,
                                   name=f"dps{ch}{j}")
                    mm(dps[0:32, :], hs, E_sb[ch][:, sl], start=True,
                       stop=True)
                    R_t = pmap.tile([32, 512], BF16, tag="R", bufs=2,
                                    name=f"R{ch}{j}")
                    V.reciprocal(out=R_t[:], in_=dps[0:32, :])
                    V.tensor_tensor(out=Ff_sb[ch][:, sl],
                                    in0=E_sb[ch][:, sl], in1=R_t[:],
                                    op=OP.mult)
                WK_sb[ch] = pmap.tile([32, N1], BF16, tag="WK", bufs=2,
                                      name=f"WK{ch}")
                for j in range(2):
                    sl = slice(512 * j, 512 * (j + 1))
                    wps = psp.tile([128, 512], F32, tag="pA", bufs=2,
                                   name=f"wps{ch}{j}")
                    mm(wps[0:32, :], avg4, Ff_sb[ch][:, sl], start=True,
                       stop=True)
                    A_.activation(out=WK_sb[ch][:, sl], in_=wps[0:32, :],
                                  func=AF.Copy)

            Gbs = {0: [], 1: []}

            def ga_dy(ch, dyi):
                Q3 = pmap.tile([32, 3, N1], BF16, tag="Q3", bufs=2,
                               name=f"Q{ch}{dyi}")
                qeng = "P" if (ch, dyi) in QT_POOL else "V"
                for j in range(2):
                    sl = slice(512 * j, 512 * (j + 1))
                    tt(qeng, Q3[:, :, sl], WWs[ch][dyi][:, :, sl],
                       WK_sb[ch][:, None, sl].broadcast_to([32, 3, 512]))
                Gb3 = pbig.tile([128, 3, N1], BF16, tag="Gb3", bufs=3,
                                name=f"Gb{ch}{dyi}")
                for dxi in range(3):
                    t = 3 * dyi + dxi
                    for j in range(2):
                        sl = slice(512 * j, 512 * (j + 1))
                        gps = psp.tile([128, 512], F32, tag="pA", bufs=2,
                                       name=f"gps{ch}{t}{j}")
                        mm(gps[:], qb, Q3[:, dxi, sl], start=True, stop=True)
                        evac(GB_EVAC[t], Gb3[:, dxi, sl], gps[:])
                Gbs[ch].append(Gb3)

            def gb_dy(ch, ob, dyi, jsplit=False):
                if dyi == 0:
                    fin_state[(ch, ob)] = psp.tile(
                        [128, N1], F32, tag="acc", bufs=2,
                        name=f"fin{ch}{ob}")
                fin = fin_state[(ch, ob)]
                o_y = 65 + 64 * (dyi - 1) - 1
                eng = "P" if (ch, ob, dyi) in FV_POOL else "V"
                Fv3 = pbig.tile([128, 3, N1], BF16, tag="Fv3", bufs=3,
                                name=f"Fv{ch}{dyi}{ob}")
                jr = range(2) if jsplit else [None]
                for jj in jr:
                    sj = slice(0, N1) if jj is None else \
                        slice(512 * jj, 512 * (jj + 1))
                    tt(eng, Fv3[:, :, sj], Gbs[ch][dyi][:, :, sj],
                       dx3(Ao_sb[ch][:, ob, o_y + sj.start :
                                      o_y + sj.start + (sj.stop - sj.start)],
                           sj.stop - sj.start))
                    for dxi in range(3):
                        for j in ([jj] if jsplit else range(2)):
                            sl = slice(512 * j, 512 * (j + 1))
                            mm(fin[:, sl], i128, Fv3[:, dxi, sl],
                               start=(dyi == 0 and dxi == 0),
                               stop=(dyi == 2 and dxi == 2))

            def gb_out(ch, ob):
                fin = fin_state[(ch, ob)]
                for j in range(2):
                    sl = slice(512 * j, 512 * (j + 1))
                    osb = pbig.tile([128, 512], BF16, tag="osb", bufs=4,
                                    name=f"osb{ch}{ob}{j}")
                    A_.activation(out=osb[:], in_=fin[:, sl], func=AF.Copy)
                    nc.gpsimd.dma_start(out=y_d[:, ch, ob, sl], in_=osb[:])

            # explicit cross-chunk interleave for engine-priority balance:
            # chunk-0's S loop outranks chunk-1's front on PE so DVE gets
            # fed while the PE grinds the second conv block.
            front(0)
            s_dy(0, 0)
            front(1)
            for dyi in (1, 2):
                s_dy(0, dyi)
            softmax(0)
            for dyi in range(3):
                ga_dy(0, dyi)
                s_dy(1, dyi)
            softmax(1)
            for dyi in range(3):
                gb_dy(0, 0, dyi, jsplit=True)
                gb_dy(0, 1, dyi, jsplit=True)
                ga_dy(1, dyi)
            gb_out(0, 0)
            gb_out(0, 1)
            for dyi in range(3):
                gb_dy(1, 0, dyi, jsplit=True)
                gb_dy(1, 1, dyi, jsplit=True)
            gb_out(1, 0)
            gb_out(1, 1)

    split_waits(nc)
    return nc


# ============================ host-side prep ===============================

def _consts():
    i32 = np.eye(32, dtype=np.float32)
    hs = np.zeros((32, 32), np.float32)
    avg4 = np.zeros((32, 32), np.float32)
    for i in range(32):
        for j in range(32):
            if i % 4 == j % 4:
                hs[i, j] = 1.0
            if i // 4 == j // 4:
                avg4[i, j] = 0.25
    qb = np.full((32, 128), 0.25, np.float32)
    i128 = np.eye(128, dtype=np.float32)
    sel = np.zeros((128, 2, 32), np.float32)
    for cb in range(2):
        for p in range(128):
            h = (128 * cb + p) // 64
            for j in range(32):
                if j % 4 == h:
                    sel[p, cb, j] = 1.0
    return i32, hs, avg4, qb, i128, sel


def _prep_inputs(x_main, x_aux, offset_w, offset_b, q_w, k_w, out_w):
    i32, hs, avg4, qb, i128, sel = _consts()
    bf16 = ml_dtypes.bfloat16
    f8 = ml_dtypes.float8_e4m3

    def wT(wmat):
        # [128, 2, 2, 128]: lhsT[cin_local, cb, ob, o_local] = w[o, cin]
        r = np.zeros((128, 2, 2, 128), np.float32)
        for cb in range(2):
            for ob in range(2):
                r[:, cb, ob, :] = wmat[128 * ob : 128 * (ob + 1),
                                       128 * cb : 128 * (cb + 1)].T
        return r

    # conv weights: fp8 scaled-residual pair; perm puts x-offset channels at
    # rows 0-7 of off, y at 8-15
    perm = [2 * k for k in range(K)] + [2 * k + 1 for k in range(K)]
    cw = np.zeros((128, 9, 2, 16), np.float32)
    for t, (dy, dx) in enumerate(TAPS):
        for cb in range(2):
            cw[:, t, cb, :] = (CSC * offset_w[perm, 128 * cb : 128 * (cb + 1),
                                              dy + 1, dx + 1]).T
    cw8 = cw.astype(f8).astype(np.float32)
    cwr = (cw - cw8).astype(f8)
    cwp = np.repeat(cw8[:, :, :, None, :], 2, axis=3).astype(f8)

    wcat = np.zeros((128, WC_W), np.float32)
    wcat[:, WC_QW : WC_QW + 512] = wT(q_w).reshape(128, 512)
    wcat[:, WC_KW : WC_KW + 512] = wT(k_w).reshape(128, 512)
    wcat[:, WC_OW : WC_OW + 512] = wT(out_w).reshape(128, 512)
    wcat[:, WC_SEL : WC_SEL + 64] = sel.reshape(128, 64)
    wcat[:, WC_I128 : WC_I128 + 128] = i128

    mcat0 = np.zeros((32, MC_W), np.float32)
    mcat0[:, MC_I32 : MC_I32 + 32] = i32
    mcat0[:, MC_HS : MC_HS + 32] = hs
    mcat0[:, MC_AVG4 : MC_AVG4 + 32] = avg4
    mcat0[:, MC_QB : MC_QB + 128] = qb
    mcat0[0:16, MC_OFFB] = offset_b[perm]
    for jj in range(32):
        mcat0[jj // 4, MC_SEL8 + jj] = 1.0           # tx: off row k
        mcat0[8 + jj // 4, MC_SEL8 + 32 + jj] = 1.0  # ty: off row 8+k

    in_maps = []
    for core in range(NCORES):
        b, half = core // 2, core % 2
        h0 = ROWS * half
        xm = np.zeros((128, 2, XM_W), np.float32)
        xa = np.zeros((128, 2, XA_W), np.float32)
        for r in range(HR):
            g = h0 - 1 + r
            if 0 <= g < H:
                for cb in range(2):
                    xm[:, cb, 2 + 66 * r : 2 + 66 * r + 64] = \
                        x_main[b, 128 * cb : 128 * (cb + 1), g, :]
                    xa[:, cb, 64 * r : 64 * r + 64] = \
                        x_aux[b, 128 * cb : 128 * (cb + 1), g, :]
        xm16 = np.zeros((128, 2, ROWS * W), np.float32)
        for cb in range(2):
            xm16[:, cb, :] = x_main[b, 128 * cb : 128 * (cb + 1),
                                    h0 : h0 + ROWS, :].reshape(128, -1)
        ycl = np.zeros((32, 4), np.float32)
        for ch in range(2):
            top, bot = h0 + CHR * ch, h0 + CHR * ch + CHR - 1
            ycl[:, 2 * ch] = -0.5 if top == 0 else -4.0
            ycl[:, 2 * ch + 1] = 0.5 if bot == H - 1 else 4.0
        x8 = xm.astype(f8).astype(np.float32)
        xq = np.stack([x8, xm - x8], axis=2).astype(f8)  # [128, 2, 2, XM_W]

        wcons = np.zeros((128, 2 * WB_W), np.uint8)
        wcons[:, 0 : 2 * WC_W] = \
            wcat.astype(bf16).view(np.uint8).reshape(128, -1)
        wcons[:, 2 * WB_CWP : 2 * WB_CWP + 576] = \
            cwp.view(np.uint8).reshape(128, -1)
        wcons[:, 2 * WB_CWR : 2 * WB_CWR + 288] = \
            cwr.view(np.uint8).reshape(128, -1)
        wcons[0:32, 2 * WB_YCL : 2 * WB_YCL + 16] = \
            ycl.view(np.uint8).reshape(32, -1)
        wcons[0:32, 2 * WB_MC : 2 * WB_MC + 2 * MC_W] = \
            mcat0.astype(bf16).view(np.uint8).reshape(32, -1)
        in_maps.append(dict(
            xq=xq, xm16=xm16.astype(bf16), xa=xa.astype(bf16),
            wcons=wcons.view(bf16)))
    return in_maps


def kernel(**inputs):
    inputs = {k: np.asarray(v, dtype=np.float32) for k, v in inputs.items()}
    if "nc" not in _CACHE:
        _CACHE["nc"] = _build_program()
    nc = _CACHE["nc"]
    in_maps = _prep_inputs(
        inputs["x_main"], inputs["x_aux"], inputs["offset_w"],
        inputs["offset_b"], inputs["q_w"], inputs["k_w"], inputs["out_w"])
    res = run_bass_kernel_spmd(nc, in_maps, list(range(NCORES))).results

    out = np.zeros((B, C, H, W), np.float32)
    for core in range(NCORES):
        b, half = core // 2, core % 2
        y = np.asarray(res[core]["y"]).astype(np.float32)  # [128, 2, 2, N1]
        for ch in range(2):
            for ob in range(2):
                out[b, 128 * ob : 128 * (ob + 1),
                    ROWS * half + CHR * ch : ROWS * half + CHR * (ch + 1),
                    :] = y[:, ch, ob, :].reshape(128, CHR, W)
    return out
